# revision 16
# baseline (speedup 1.0000x reference)
"""Multi-head causal attention (B=2, S=2048, D=1024, H=16) on 8 NeuronCores.

Sharding: core c handles batch c//4 and head group c%4 (4 heads = 256 dims).
Wq/Wk/Wv are column-split by head; Wo is row-split; the Wo all-reduce is done
on the host by summing the 8 partial outputs (4 cores per batch), plus bo.

Key design points:
  - ALL matmuls occupy the full 128x128 PE array. The HAM clock gate only
    counts full-array activity as "busy": half-array matmuls (K=64 scores,
    M=65 attn@V) leave the PE throttled at 1.2 GHz forever. Scores use a
    zero-padded kTz (head's 64 rows in its partition half, zeros elsewhere)
    against the full 128-partition qT; attn@V uses voz padded to 128 columns
    (64 V dims + ones column + 63 zeros). Zero rows/columns add no stream
    cycles - the padding is free.
  - Phase B runs single-head streams, exp-paced on ScalarE (the hard floor:
    ~81us of exp). PSUM: scores double-buffered (4 banks) + 2 oT tiles
    (4 banks) = 8. Warm PE work per kt (~854ns) hides under exp (~1075ns).
  - attn@V deferred 2 kts behind scores so the GpSimd tri-mask multiply
    never stalls the PE.
  - The rowsum->reciprocal->broadcast chain (4 serial DMAs of latency)
    starts at its own head's end and is software-pipelined at EMISSION
    level: remaining steps are emitted at kt milestones inside the next
    head's loop (final chains inside phase C), so no in-order engine queue
    ever blocks on a chain dependency.
  - bf16 everywhere SBUF-resident (fp8 fails accuracy: quantization errors
    scale with the result, they do not average down); rowsum chain f32.
  - Input chunks round-robin across all 3 DMA-capable queues (SP/Act/Pool)
    so each projection streams at aggregate bandwidth.
  - qh=1 processed before qh=0 so phase C's qt 8-15 (which need only qh=1
    norms) stream immediately after B's last matmul.
  - PE warm-up matmuls during the initial input DMA wait (HAM ramp).
  - y written bf16 (halves output DMA); host accumulates in f32.
"""

import ml_dtypes
import numpy as np

import concourse.bass as bass
import concourse.tile as tile
from concourse import bacc, mybir
from concourse.bass_utils import run_bass_kernel_spmd

B, S, D, H = 2, 2048, 1024, 16
DH = D // H          # 64
HPC = 4              # heads per core
HD = HPC * DH        # 256 head dims per core
N_CORES = 8
DT = D // 128        # 8 contraction tiles for projections
NT = S // 128        # 16 seq tiles
F32 = mybir.dt.float32
BF16 = mybir.dt.bfloat16


def build_program():
    nc = bacc.Bacc("TRN2", target_bir_lowering=False, debug=False,
                   num_devices=N_CORES)

    qT_d = nc.dram_tensor("qT", [D, S], BF16, kind="ExternalInput").ap()
    kT_d = nc.dram_tensor("kT", [D, S], BF16, kind="ExternalInput").ap()
    vT_d = nc.dram_tensor("vT", [D, S], BF16, kind="ExternalInput").ap()
    # weights pre-transposed on host to [128, dt*128|t*512] so each DMA is
    # one 4KB-contiguous line per partition (the (t p) h rearrange view has
    # 512-byte lines = 1024 descriptors = slow)
    wq_d = nc.dram_tensor("wq", [128, DT * HD], BF16,
                          kind="ExternalInput").ap()
    wk_d = nc.dram_tensor("wk", [128, DT * HD], BF16,
                          kind="ExternalInput").ap()
    wv_d = nc.dram_tensor("wv", [128, DT * HD], BF16,
                          kind="ExternalInput").ap()
    wo_d = nc.dram_tensor("wo", [128, 2 * D], BF16, kind="ExternalInput").ap()
    tri_d = nc.dram_tensor("tri", [128, 128], BF16, kind="ExternalInput").ap()
    ones_d = nc.dram_tensor("ones", [128, 64], BF16,
                            kind="ExternalInput").ap()
    # DRAM staging for rowsums and their reciprocals (partition remapping
    # and step-0 broadcasts are only possible through DRAM)
    rb_d = nc.dram_tensor("rbounce", [16, 1024], F32).ap()
    rrec_d = nc.dram_tensor("rrec", [16, 1024], F32).ap()
    y_d = nc.dram_tensor("y", [S, D], BF16, kind="ExternalOutput").ap()

    Exp = mybir.ActivationFunctionType.Exp
    DMAE = None  # set inside context

    with tile.TileContext(nc) as tc:
        dma_engs = (nc.sync, nc.gpsimd, nc.scalar)
        with (
            tc.tile_pool(name="persist", bufs=1) as persist,
            tc.tile_pool(name="qstream", bufs=3) as qstream,
            tc.tile_pool(name="kstream", bufs=3) as kstream,
            tc.tile_pool(name="vstream", bufs=3) as vstream,
            tc.tile_pool(name="pT", bufs=5) as pT_pool,
            tc.tile_pool(name="norm", bufs=2) as norm_pool,
            tc.tile_pool(name="ysb", bufs=6) as ysb_pool,
        ):
            # ---- constants / weights (single-shot DMAs, tiny first) ----
            tri_sb = persist.tile([128, 128], BF16, tag="tri")
            ones_sb = persist.tile([128, 64], BF16, tag="ones")
            nc.sync.dma_start(tri_sb, tri_d)
            nc.sync.dma_start(ones_sb, ones_d)

            wq_sb = persist.tile([128, DT, HD], BF16, tag="wq")
            wk_sb = persist.tile([128, DT, HD], BF16, tag="wk")
            wv_sb = persist.tile([128, DT, HD], BF16, tag="wv")
            wo_sb = persist.tile([128, 2, D], BF16, tag="wo")
            nc.sync.dma_start(wq_sb, wq_d.rearrange("p (t h) -> p t h", t=DT))
            nc.gpsimd.dma_start(
                wk_sb, wk_d.rearrange("p (t h) -> p t h", t=DT))
            nc.scalar.dma_start(
                wv_sb, wv_d.rearrange("p (t h) -> p t h", t=DT))

            qT_sb = persist.tile([128, 2, S], BF16, tag="qTsb")
            # kTz: per head parity e, the head's 64 kT rows sit in its
            # partition half with ZEROS in the other half -> scores contract
            # K=128 (full array) against the full-partition qT; the other
            # head's q rows are multiplied by zero weights.
            kTz_sb = persist.tile([128, 2, 2, S], BF16, tag="kTz")
            nc.vector.memset(kTz_sb[64:128, 0, :, :], 0.0)
            nc.vector.memset(kTz_sb[0:64, 1, :, :], 0.0)
            # voz: V (64 dims) + ones column (index 64, so attn@V also emits
            # the softmax row-sum as output row 64) + 63 zero columns so the
            # stationary operand is full 128 wide.
            voz_sb = persist.tile([128, NT, HPC, 128], BF16, tag="voz")
            nc.vector.memset(voz_sb[:, :, :, DH + 1:], 0.0)
            for h in range(HPC):
                nc.vector.tensor_copy(
                    voz_sb[:, :, h, DH:DH + 1],
                    ones_sb[:, 0:NT].rearrange("p (t o) -> p t o", o=1))
            aT_sb = persist.tile([128, 2, S], BF16, tag="aTsb")

            # ---- PE warm-up during initial DMA wait (HAM ramp) ----
            with tc.tile_pool(name="warm", bufs=1, space="PSUM") as warmp:
                wps = warmp.tile([128, 128], F32, tag="warm")
                for _ in range(52):
                    nc.tensor.matmul(wps, tri_sb, tri_sb,
                                     start=True, stop=True)
            # preload the exp activation table off the critical path
            dummy_pt = persist.tile([128, 16], BF16, tag="dummy")
            nc.scalar.activation(dummy_pt, tri_sb[:, 0:16], Exp, scale=0.125)

            # ---- Phase A: projections ----
            with tc.tile_pool(name="psA", bufs=8, space="PSUM") as psA:
                def proj_qk(x_d, w_sb, evac, pool):
                    ps = [psA.tile([128, 512], F32, tag="psA", name=f"psA{i}")
                          for i in range(8)]
                    for dt in range(DT):
                        # one full-row chunk per dt: 4KB lines, one issue
                        xt = pool.tile([128, S], BF16)
                        dma_engs[dt % 3].dma_start(
                            xt, x_d[dt * 128:(dt + 1) * 128, :])
                        for t in range(2):
                            for cc in range(4):
                                nc.tensor.matmul(
                                    ps[t * 4 + cc],
                                    w_sb[:, dt, t * 128:(t + 1) * 128],
                                    xt[:, cc * 512:(cc + 1) * 512],
                                    start=(dt == 0), stop=(dt == DT - 1),
                                )
                    for t in range(2):
                        for c in range(4):
                            evac(t, c, ps[t * 4 + c])

                def q_evac(t, c, src):
                    nc.vector.tensor_copy(
                        qT_sb[:, t, c * 512:(c + 1) * 512], src)

                def k_evac(t, c, src):
                    # head-even rows (0:64) -> kTz parity 0, lanes 0:64;
                    # head-odd rows (64:128) -> kTz parity 1, lanes 64:128.
                    nc.vector.tensor_copy(
                        kTz_sb[0:64, 0, t, c * 512:(c + 1) * 512],
                        src[0:64, :])
                    nc.vector.tensor_copy(
                        kTz_sb[64:128, 1, t, c * 512:(c + 1) * 512],
                        src[64:128, :])

                proj_qk(qT_d, wq_sb, q_evac, qstream)
                proj_qk(kT_d, wk_sb, k_evac, kstream)

                # V: natural layout out[m = seq_tile(128), n = head dims(256)]
                psv = [psA.tile([128, 512], F32, tag="psA", name=f"psV{i}")
                       for i in range(8)]
                for dt in range(DT):
                    vt = vstream.tile([128, S], BF16)
                    dma_engs[dt % 3].dma_start(
                        vt, vT_d[dt * 128:(dt + 1) * 128, :])
                    for ntile in range(NT):
                        nc.tensor.matmul(
                            psv[ntile // 2][:,
                                            (ntile % 2) * 256:(ntile % 2) * 256 + 256],
                            vt[:, ntile * 128:(ntile + 1) * 128],
                            wv_sb[:, dt, :],
                            # start only on the bank's first matmul:
                            # start=True clears has_written for the WHOLE
                            # bank; the second group must not re-clear.
                            start=(dt == 0 and ntile % 2 == 0),
                            stop=(dt == DT - 1),
                            skip_group_check=True,
                        )
                for ntile in range(NT):
                    src = psv[ntile // 2][:,
                                          (ntile % 2) * 256:(ntile % 2) * 256 + 256]
                    nc.vector.tensor_copy(
                        voz_sb[:, ntile, :, 0:DH],
                        src.rearrange("p (h d) -> p h d", h=HPC),
                    )

            # ---- Phase B: attention, single-head streams, exp-paced ----
            def make_chain(t, p64, qh0, oT, i):
                """Normalization chain for one finished head: aT[head rows]
                = oT * (1/rowsum). s1 is emitted at the head's own end; the
                rest at kt milestones of the next head (or phase C blocks)
                so no in-order engine queue blocks on chain DMA latency."""
                rs = norm_pool.tile([65, 1024], F32, tag="rs", name="rs")
                r64 = norm_pool.tile([64, 16], F32, tag="r64", name="r64")
                rec64 = norm_pool.tile([64, 16], F32, tag="rec64",
                                       name="rec64")
                rb = norm_pool.tile([64, 1024], F32, tag="rb", name="rb")
                row = rb_d[i:i + 1, :]
                rrow = rrec_d[i:i + 1, :]

                def s1():
                    nc.vector.tensor_copy(rs[64:65, :], oT[64:65, :])
                    nc.sync.dma_start(row, rs[64:65, :])

                def s2():
                    r64view = bass.AP(tensor=row.tensor, offset=row.offset,
                                      ap=[[16, 64], [1, 16]])
                    nc.gpsimd.dma_start(r64, r64view)

                def s3():
                    nc.vector.reciprocal(rec64, r64)
                    recback = bass.AP(tensor=rrow.tensor, offset=rrow.offset,
                                      ap=[[16, 64], [1, 16]])
                    nc.sync.dma_start(recback, rec64)

                def s4():
                    bcast = bass.AP(tensor=rrow.tensor, offset=rrow.offset,
                                    ap=[[0, 64]] + [list(rrow.ap[-1])])
                    nc.gpsimd.dma_start(rb, bcast)

                def s5():
                    if p64 == 0:
                        nc.vector.tensor_mul(
                            aT_sb[0:64, t, qh0:qh0 + 1024], oT[0:64, :], rb)
                    else:
                        stage = norm_pool.tile([64, 1024], BF16, tag="stage",
                                               name="stage")
                        nc.vector.tensor_mul(stage, oT[0:64, :], rb)
                        nc.gpsimd.dma_start(
                            aT_sb[64:128, t, qh0:qh0 + 1024], stage)

                s1()
                return [s2, s3, s4, s5]

            with (
                tc.tile_pool(name="psB", bufs=2, space="PSUM") as psB,
                tc.tile_pool(name="psO", bufs=2, space="PSUM") as psO,
            ):
                nc.sync.dma_start(
                    wo_sb, wo_d.rearrange("p (t e) -> p t e", t=2))

                chain_i = 0
                chain_ops = []  # pending emission closures from prior heads
                # Interleave qh=0 (8-kt) and qh=1 (16-kt) streams so every
                # norm chain has >=21us of following stream to drain under
                # (a qh=0 head alone is 6us < the ~9us chain latency).
                # Last stream is qh=1 h3, so phase C runs qt 0-7 first.
                for qh, hh in ((0, 0), (1, 0), (0, 1), (1, 1),
                               (0, 2), (1, 2), (0, 3), (1, 3)):
                    qh0 = qh * 1024
                    nkt = (qh0 + 1024) // 128
                    # kt milestones at which to emit one pending chain step
                    if nkt == 8:
                        miles = {1, 3, 5, 7}
                    else:
                        miles = {2, 5, 8, 11}
                    if True:
                        t, e = hh // 2, hh % 2
                        p64 = e * 64
                        oT = psO.tile([128, 1024], F32, tag="oT", name="oT")

                        def attn_v(kt, pT_t, qs, oT=oT, qh0=qh0, hh=hh):
                            for qc in range(2):
                                c0 = qh0 + qc * 512
                                lo = max(qs, c0)
                                if lo >= c0 + 512:
                                    continue
                                ktl = (c0 + 512) // 128 - 1
                                nc.tensor.matmul(
                                    oT[:, qc * 512 + (lo - c0):(qc + 1) * 512],
                                    voz_sb[:, kt, hh, :],
                                    pT_t[:, lo - qh0:c0 + 512 - qh0],
                                    start=(kt == 0), stop=(kt == ktl),
                                )

                        pending = []
                        for kt in range(nkt):
                            k0 = kt * 128
                            qs = max(k0, qh0)
                            off = qs - qh0
                            st = psB.tile([128, 1024], F32, tag="st",
                                          name="st")
                            for bank in range(2):
                                glo = max(qs, qh0 + bank * 512)
                                ghi = qh0 + (bank + 1) * 512
                                if glo < ghi:
                                    nc.tensor.matmul(
                                        st[:, glo - qh0:ghi - qh0],
                                        kTz_sb[:, e, t, k0:k0 + 128],
                                        qT_sb[:, t, glo:ghi],
                                        start=True, stop=True,
                                    )
                            pT_t = pT_pool.tile([128, 1024], BF16, tag="pT",
                                                name="pT")
                            nc.scalar.activation(
                                pT_t[:, off:1024], st[:, off:1024], Exp,
                                scale=0.125)
                            if k0 >= qh0:
                                nc.gpsimd.tensor_mul(
                                    pT_t[:, off:off + 128],
                                    pT_t[:, off:off + 128], tri_sb)
                            pending.append((kt, pT_t, qs))
                            if len(pending) > 2:
                                attn_v(*pending.pop(0))
                            if kt in miles and chain_ops:
                                chain_ops.pop(0)()
                        for p_ in pending:
                            attn_v(*p_)
                        # flush any chain debt before enqueueing this head's
                        for op in chain_ops:
                            op()
                        chain_ops = make_chain(t, p64, qh0, oT, chain_i)
                        chain_i += 1

                # ---- Phase C: output projection, interleaved with the
                # ---- final (qh=1 h3) chain. qt 0-7 first: they need only
                # ---- qh=0 norms, which completed long ago. PSUM comes
                # ---- from the scores pool (tag st) - no spare banks.
                for i, qt in enumerate(list(range(8)) + list(range(8, NT))):
                    if i in (0, 1, 2, 3) and chain_ops:
                        chain_ops.pop(0)()
                    for e in range(2):
                        py = psB.tile([128, 1024], F32, tag="st", name="py")
                        for t in range(2):
                            nc.tensor.matmul(
                                py[:, 0:512],
                                aT_sb[:, t, qt * 128:(qt + 1) * 128],
                                wo_sb[:, t, e * 512:(e + 1) * 512],
                                start=(t == 0), stop=(t == 1),
                            )
                        ot = ysb_pool.tile([128, 512], BF16, tag="ysb",
                                           name="ysb")
                        if (i * 2 + e) % 2 == 0:
                            nc.vector.tensor_copy(ot, py[:, 0:512])
                        else:
                            nc.scalar.copy(ot, py[:, 0:512])
                        dma_eng = dma_engs[(i * 2 + e) % 3]
                        dma_eng.dma_start(
                            y_d[qt * 128:(qt + 1) * 128,
                                e * 512:(e + 1) * 512], ot)

    nc.compile()
    return nc


_CACHE = {}
last_in_maps = None


def _get_program():
    if "nc" not in _CACHE:
        _CACHE["nc"] = build_program()
    return _CACHE["nc"]


def kernel(query, key, value, mask, Wq, Wk, Wv, Wo, bo):
    query = np.asarray(query, np.float32)
    key = np.asarray(key, np.float32)
    value = np.asarray(value, np.float32)
    Wq = np.asarray(Wq, np.float32)
    Wk = np.asarray(Wk, np.float32)
    Wv = np.asarray(Wv, np.float32)
    Wo = np.asarray(Wo, np.float32)
    bo = np.asarray(bo, np.float32)

    nc = _get_program()
    tri = np.ascontiguousarray(np.triu(np.ones((128, 128), np.float32)))
    WoT = Wo.T  # (d_in, d_out)

    def wlayout(w):  # [(t 128), n] -> [128, t*n]: partition p holds t*128+p
        return np.ascontiguousarray(
            w.reshape(w.shape[0] // 128, 128, -1)
            .transpose(1, 0, 2).reshape(128, -1))

    in_maps = []
    for c in range(N_CORES):
        b, g = divmod(c, 4)
        hs = slice(g * HD, (g + 1) * HD)
        in_maps.append({
            "qT": np.ascontiguousarray(query[b].T).astype(ml_dtypes.bfloat16),
            "kT": np.ascontiguousarray(key[b].T).astype(ml_dtypes.bfloat16),
            "vT": np.ascontiguousarray(value[b].T).astype(ml_dtypes.bfloat16),
            "wq": wlayout(Wq[hs].T).astype(ml_dtypes.bfloat16),
            "wk": wlayout(Wk[hs].T).astype(ml_dtypes.bfloat16),
            "wv": wlayout(Wv[hs].T).astype(ml_dtypes.bfloat16),
            "wo": wlayout(WoT[hs]).astype(ml_dtypes.bfloat16),
            "tri": tri.astype(ml_dtypes.bfloat16),
            "ones": np.ones((128, 64), ml_dtypes.bfloat16),
        })

    global last_in_maps
    last_in_maps = in_maps
    res = run_bass_kernel_spmd(nc, in_maps, core_ids=list(range(N_CORES)))

    out = np.zeros((B, S, D), np.float32)
    for c in range(N_CORES):
        out[c // 4] += np.asarray(res.results[c]["y"], np.float32)
    out += bo
    return out


# revision 20
# speedup vs baseline: 1.0482x; 1.0482x over previous
"""Multi-head causal attention (B=2, S=2048, D=1024, H=16) on 8 NeuronCores.

Sharding: core c handles batch c//4 and head group c%4 (4 heads = 256 dims).
Wq/Wk/Wv are column-split by head; Wo is row-split; the Wo all-reduce is done
on the host by summing the 8 partial outputs (4 cores per batch), plus bo.

Key design points:
  - ALL matmuls occupy the full 128x128 PE array. The HAM clock gate only
    counts full-array activity as "busy": half-array matmuls (K=64 scores,
    M=65 attn@V) leave the PE throttled at 1.2 GHz forever. Scores use a
    zero-padded kTz (head's 64 rows in its partition half, zeros elsewhere)
    against the full 128-partition qT; attn@V uses voz padded to 128 columns
    (64 V dims + ones column + 63 zeros). Zero rows/columns add no stream
    cycles - the padding is free.
  - Phase B runs single-head streams, exp-paced on ScalarE (the hard floor:
    ~81us of exp). PSUM: scores double-buffered (4 banks) + 2 oT tiles
    (4 banks) = 8. Warm PE work per kt (~854ns) hides under exp (~1075ns).
  - attn@V deferred 2 kts behind scores so the GpSimd tri-mask multiply
    never stalls the PE.
  - The rowsum->reciprocal->broadcast chain (4 serial DMAs of latency)
    starts at its own head's end and is software-pipelined at EMISSION
    level: remaining steps are emitted at kt milestones inside the next
    head's loop (final chains inside phase C), so no in-order engine queue
    ever blocks on a chain dependency.
  - bf16 everywhere SBUF-resident (fp8 fails accuracy: quantization errors
    scale with the result, they do not average down); rowsum chain f32.
  - Input chunks round-robin across all 3 DMA-capable queues (SP/Act/Pool)
    so each projection streams at aggregate bandwidth.
  - qh=1 processed before qh=0 so phase C's qt 8-15 (which need only qh=1
    norms) stream immediately after B's last matmul.
  - PE warm-up matmuls during the initial input DMA wait (HAM ramp).
  - y written bf16 (halves output DMA); host accumulates in f32.
"""

import ml_dtypes
import numpy as np

import concourse.bass as bass
import concourse.tile as tile
from concourse import bacc, mybir
from concourse.bass_utils import run_bass_kernel_spmd

B, S, D, H = 2, 2048, 1024, 16
DH = D // H          # 64
HPC = 4              # heads per core
HD = HPC * DH        # 256 head dims per core
N_CORES = 8
DT = D // 128        # 8 contraction tiles for projections
NT = S // 128        # 16 seq tiles
F32 = mybir.dt.float32
BF16 = mybir.dt.bfloat16


def build_program():
    nc = bacc.Bacc("TRN2", target_bir_lowering=False, debug=False,
                   num_devices=N_CORES)

    qT_d = nc.dram_tensor("qT", [D, S], BF16, kind="ExternalInput").ap()
    kT_d = nc.dram_tensor("kT", [D, S], BF16, kind="ExternalInput").ap()
    vT_d = nc.dram_tensor("vT", [D, S], BF16, kind="ExternalInput").ap()
    # weights pre-transposed on host to [128, dt*128|t*512] so each DMA is
    # one 4KB-contiguous line per partition (the (t p) h rearrange view has
    # 512-byte lines = 1024 descriptors = slow)
    wq_d = nc.dram_tensor("wq", [128, DT * HD], BF16,
                          kind="ExternalInput").ap()
    wk_d = nc.dram_tensor("wk", [128, DT * HD], BF16,
                          kind="ExternalInput").ap()
    wv_d = nc.dram_tensor("wv", [128, DT * HD], BF16,
                          kind="ExternalInput").ap()
    wo_d = nc.dram_tensor("wo", [128, 2 * D], BF16, kind="ExternalInput").ap()
    tri_d = nc.dram_tensor("tri", [128, 128], BF16, kind="ExternalInput").ap()
    ones_d = nc.dram_tensor("ones", [128, 64], BF16,
                            kind="ExternalInput").ap()
    # DRAM staging for rowsums and their reciprocals (partition remapping
    # and step-0 broadcasts are only possible through DRAM)
    rb_d = nc.dram_tensor("rbounce", [16, 1024], F32).ap()
    rrec_d = nc.dram_tensor("rrec", [16, 1024], F32).ap()
    y_d = nc.dram_tensor("y", [S, D], BF16, kind="ExternalOutput").ap()

    Exp = mybir.ActivationFunctionType.Exp
    DMAE = None  # set inside context

    with tile.TileContext(nc) as tc:
        dma_engs = (nc.sync, nc.gpsimd, nc.scalar)
        with (
            tc.tile_pool(name="persist", bufs=1) as persist,
            tc.tile_pool(name="qstream", bufs=6) as qstream,
            tc.tile_pool(name="kstream", bufs=6) as kstream,
            tc.tile_pool(name="vstream", bufs=6) as vstream,
            tc.tile_pool(name="pT", bufs=5) as pT_pool,
            tc.tile_pool(name="norm", bufs=2) as norm_pool,
            tc.tile_pool(name="ysb", bufs=6) as ysb_pool,
        ):
            # ---- constants / weights (single-shot DMAs, tiny first) ----
            tri_sb = persist.tile([128, 128], BF16, tag="tri")
            ones_sb = persist.tile([128, 64], BF16, tag="ones")
            nc.sync.dma_start(tri_sb, tri_d)
            nc.sync.dma_start(ones_sb, ones_d)

            wq_sb = persist.tile([128, DT, HD], BF16, tag="wq")
            wk_sb = persist.tile([128, DT, HD], BF16, tag="wk")
            wv_sb = persist.tile([128, DT, HD], BF16, tag="wv")
            wo_sb = persist.tile([128, 2, D], BF16, tag="wo")
            nc.sync.dma_start(wq_sb, wq_d.rearrange("p (t h) -> p t h", t=DT))
            nc.gpsimd.dma_start(
                wk_sb, wk_d.rearrange("p (t h) -> p t h", t=DT))
            nc.scalar.dma_start(
                wv_sb, wv_d.rearrange("p (t h) -> p t h", t=DT))

            qT_sb = persist.tile([128, 2, S], BF16, tag="qTsb")
            # kTz: per head parity e, the head's 64 kT rows sit in its
            # partition half with ZEROS in the other half -> scores contract
            # K=128 (full array) against the full-partition qT; the other
            # head's q rows are multiplied by zero weights.
            kTz_sb = persist.tile([128, 2, 2, S], BF16, tag="kTz")
            nc.vector.memset(kTz_sb[64:128, 0, :, :], 0.0)
            nc.vector.memset(kTz_sb[0:64, 1, :, :], 0.0)
            # voz: V (64 dims) + ones column (index 64, so attn@V also emits
            # the softmax row-sum as output row 64) + 63 zero columns so the
            # stationary operand is full 128 wide.
            voz_sb = persist.tile([128, NT, HPC, 128], BF16, tag="voz")
            nc.vector.memset(voz_sb[:, :, :, DH + 1:], 0.0)
            for h in range(HPC):
                nc.vector.tensor_copy(
                    voz_sb[:, :, h, DH:DH + 1],
                    ones_sb[:, 0:NT].rearrange("p (t o) -> p t o", o=1))
            aT_sb = persist.tile([128, 2, S], BF16, tag="aTsb")

            # ---- PE warm-up during initial DMA wait (HAM ramp) ----
            with tc.tile_pool(name="warm", bufs=1, space="PSUM") as warmp:
                wps = warmp.tile([128, 128], F32, tag="warm")
                for _ in range(52):
                    nc.tensor.matmul(wps, tri_sb, tri_sb,
                                     start=True, stop=True)
            # preload the exp activation table off the critical path
            dummy_pt = persist.tile([128, 16], BF16, tag="dummy")
            nc.scalar.activation(dummy_pt, tri_sb[:, 0:16], Exp, scale=0.125)

            # ---- Phase A: projections ----
            with tc.tile_pool(name="psA", bufs=8, space="PSUM") as psA:
                def proj_qk(x_d, w_sb, evac, pool):
                    ps = [psA.tile([128, 512], F32, tag="psA", name=f"psA{i}")
                          for i in range(8)]
                    for dt in range(DT):
                        for half in range(2):
                            xt = pool.tile([128, S // 2], BF16)
                            dma_engs[(dt * 2 + half) % 3].dma_start(
                                xt, x_d[dt * 128:(dt + 1) * 128,
                                        half * 1024:(half + 1) * 1024])
                            for t in range(2):
                                for c in range(2):
                                    cc = half * 2 + c
                                    nc.tensor.matmul(
                                        ps[t * 4 + cc],
                                        w_sb[:, dt, t * 128:(t + 1) * 128],
                                        xt[:, c * 512:(c + 1) * 512],
                                        start=(dt == 0), stop=(dt == DT - 1),
                                    )
                    for t in range(2):
                        for c in range(4):
                            evac(t, c, ps[t * 4 + c])

                def q_evac(t, c, src):
                    nc.vector.tensor_copy(
                        qT_sb[:, t, c * 512:(c + 1) * 512], src)

                def k_evac(t, c, src):
                    # head-even rows (0:64) -> kTz parity 0, lanes 0:64;
                    # head-odd rows (64:128) -> kTz parity 1, lanes 64:128.
                    nc.vector.tensor_copy(
                        kTz_sb[0:64, 0, t, c * 512:(c + 1) * 512],
                        src[0:64, :])
                    nc.vector.tensor_copy(
                        kTz_sb[64:128, 1, t, c * 512:(c + 1) * 512],
                        src[64:128, :])

                proj_qk(qT_d, wq_sb, q_evac, qstream)
                proj_qk(kT_d, wk_sb, k_evac, kstream)

                # V: natural layout out[m = seq_tile(128), n = head dims(256)]
                psv = [psA.tile([128, 512], F32, tag="psA", name=f"psV{i}")
                       for i in range(8)]
                for dt in range(DT):
                    for half in range(2):
                        vt = vstream.tile([128, S // 2], BF16)
                        dma_engs[(dt * 2 + half) % 3].dma_start(
                            vt, vT_d[dt * 128:(dt + 1) * 128,
                                     half * 1024:(half + 1) * 1024])
                        for nt_ in range(8):
                            ntile = half * 8 + nt_
                            nc.tensor.matmul(
                                psv[ntile // 2][:,
                                                (ntile % 2) * 256:(ntile % 2) * 256 + 256],
                                vt[:, nt_ * 128:(nt_ + 1) * 128],
                                wv_sb[:, dt, :],
                                # start only on the bank's first matmul:
                                # start=True clears has_written for the WHOLE
                                # bank; the second group must not re-clear.
                                start=(dt == 0 and ntile % 2 == 0),
                                stop=(dt == DT - 1),
                                skip_group_check=True,
                            )
                for ntile in range(NT):
                    src = psv[ntile // 2][:,
                                          (ntile % 2) * 256:(ntile % 2) * 256 + 256]
                    nc.vector.tensor_copy(
                        voz_sb[:, ntile, :, 0:DH],
                        src.rearrange("p (h d) -> p h d", h=HPC),
                    )

            # ---- Phase B: attention, single-head streams, exp-paced ----
            def make_chain(t, p64, qh0, oT, i):
                """Normalization chain for one finished head: aT[head rows]
                = oT * (1/rowsum). s1 is emitted at the head's own end; the
                rest at kt milestones of the next head (or phase C blocks)
                so no in-order engine queue blocks on chain DMA latency."""
                rs = norm_pool.tile([65, 1024], F32, tag="rs", name="rs")
                r64 = norm_pool.tile([64, 16], F32, tag="r64", name="r64")
                rec64 = norm_pool.tile([64, 16], F32, tag="rec64",
                                       name="rec64")
                rb = norm_pool.tile([64, 1024], F32, tag="rb", name="rb")
                row = rb_d[i:i + 1, :]
                rrow = rrec_d[i:i + 1, :]

                def s1():
                    nc.vector.tensor_copy(rs[64:65, :], oT[64:65, :])
                    nc.sync.dma_start(row, rs[64:65, :])

                def s2():
                    r64view = bass.AP(tensor=row.tensor, offset=row.offset,
                                      ap=[[16, 64], [1, 16]])
                    nc.gpsimd.dma_start(r64, r64view)

                def s3():
                    nc.vector.reciprocal(rec64, r64)
                    recback = bass.AP(tensor=rrow.tensor, offset=rrow.offset,
                                      ap=[[16, 64], [1, 16]])
                    nc.sync.dma_start(recback, rec64)

                def s4():
                    bcast = bass.AP(tensor=rrow.tensor, offset=rrow.offset,
                                    ap=[[0, 64]] + [list(rrow.ap[-1])])
                    nc.gpsimd.dma_start(rb, bcast)

                def s5():
                    if p64 == 0:
                        nc.vector.tensor_mul(
                            aT_sb[0:64, t, qh0:qh0 + 1024], oT[0:64, :], rb)
                    else:
                        stage = norm_pool.tile([64, 1024], BF16, tag="stage",
                                               name="stage")
                        nc.vector.tensor_mul(stage, oT[0:64, :], rb)
                        nc.gpsimd.dma_start(
                            aT_sb[64:128, t, qh0:qh0 + 1024], stage)

                s1()
                return [s2, s3, s4, s5]

            with (
                tc.tile_pool(name="psB", bufs=2, space="PSUM") as psB,
                tc.tile_pool(name="psO", bufs=2, space="PSUM") as psO,
            ):
                nc.sync.dma_start(
                    wo_sb, wo_d.rearrange("p (t e) -> p t e", t=2))

                chain_i = 0
                chain_ops = []  # pending emission closures from prior heads
                # qh=0 heads first: their short streams (low PE duty) run
                # in one contiguous block; the long qh=1 streams then hold
                # the HAM clock warm and every chain has a >=15us stream
                # behind it to drain under. Last stream is qh=1 h3, so
                # phase C runs qt 0-7 (qh=0-dependent) first.
                for qh, hh in ((0, 0), (0, 1), (0, 2), (0, 3),
                               (1, 0), (1, 1), (1, 2), (1, 3)):
                    qh0 = qh * 1024
                    nkt = (qh0 + 1024) // 128
                    # kt milestones at which to emit one pending chain step
                    if nkt == 8:
                        miles = {1, 3, 5, 7}
                    else:
                        miles = {2, 5, 8, 11}
                    if True:
                        t, e = hh // 2, hh % 2
                        p64 = e * 64
                        oT = psO.tile([128, 1024], F32, tag="oT", name="oT")

                        def attn_v(kt, pT_t, qs, oT=oT, qh0=qh0, hh=hh):
                            for qc in range(2):
                                c0 = qh0 + qc * 512
                                lo = max(qs, c0)
                                if lo >= c0 + 512:
                                    continue
                                ktl = (c0 + 512) // 128 - 1
                                nc.tensor.matmul(
                                    oT[:, qc * 512 + (lo - c0):(qc + 1) * 512],
                                    voz_sb[:, kt, hh, :],
                                    pT_t[:, lo - qh0:c0 + 512 - qh0],
                                    start=(kt == 0), stop=(kt == ktl),
                                )

                        pending = []
                        for kt in range(nkt):
                            k0 = kt * 128
                            qs = max(k0, qh0)
                            off = qs - qh0
                            st = psB.tile([128, 1024], F32, tag="st",
                                          name="st")
                            for bank in range(2):
                                glo = max(qs, qh0 + bank * 512)
                                ghi = qh0 + (bank + 1) * 512
                                if glo < ghi:
                                    nc.tensor.matmul(
                                        st[:, glo - qh0:ghi - qh0],
                                        kTz_sb[:, e, t, k0:k0 + 128],
                                        qT_sb[:, t, glo:ghi],
                                        start=True, stop=True,
                                    )
                            pT_t = pT_pool.tile([128, 1024], BF16, tag="pT",
                                                name="pT")
                            nc.scalar.activation(
                                pT_t[:, off:1024], st[:, off:1024], Exp,
                                scale=0.125)
                            if k0 >= qh0:
                                nc.gpsimd.tensor_mul(
                                    pT_t[:, off:off + 128],
                                    pT_t[:, off:off + 128], tri_sb)
                            pending.append((kt, pT_t, qs))
                            if len(pending) > 2:
                                attn_v(*pending.pop(0))
                            if kt in miles and chain_ops:
                                chain_ops.pop(0)()
                        for p_ in pending:
                            attn_v(*p_)
                        # flush any chain debt before enqueueing this head's
                        for op in chain_ops:
                            op()
                        chain_ops = make_chain(t, p64, qh0, oT, chain_i)
                        chain_i += 1

                # ---- Phase C: output projection, interleaved with the
                # ---- final (qh=1 h3) chain. qt 0-7 first: they need only
                # ---- qh=0 norms, which completed long ago. PSUM comes
                # ---- from the scores pool (tag st) - no spare banks.
                for i, qt in enumerate(list(range(8)) + list(range(8, NT))):
                    if i in (0, 1, 2, 3) and chain_ops:
                        chain_ops.pop(0)()
                    for e in range(2):
                        py = psB.tile([128, 1024], F32, tag="st", name="py")
                        for t in range(2):
                            nc.tensor.matmul(
                                py[:, 0:512],
                                aT_sb[:, t, qt * 128:(qt + 1) * 128],
                                wo_sb[:, t, e * 512:(e + 1) * 512],
                                start=(t == 0), stop=(t == 1),
                            )
                        ot = ysb_pool.tile([128, 512], BF16, tag="ysb",
                                           name="ysb")
                        if (i * 2 + e) % 2 == 0:
                            nc.vector.tensor_copy(ot, py[:, 0:512])
                        else:
                            nc.scalar.copy(ot, py[:, 0:512])
                        dma_eng = dma_engs[(i * 2 + e) % 3]
                        dma_eng.dma_start(
                            y_d[qt * 128:(qt + 1) * 128,
                                e * 512:(e + 1) * 512], ot)

    nc.compile()
    return nc


_CACHE = {}
last_in_maps = None


def _get_program():
    if "nc" not in _CACHE:
        _CACHE["nc"] = build_program()
    return _CACHE["nc"]


def kernel(query, key, value, mask, Wq, Wk, Wv, Wo, bo):
    query = np.asarray(query, np.float32)
    key = np.asarray(key, np.float32)
    value = np.asarray(value, np.float32)
    Wq = np.asarray(Wq, np.float32)
    Wk = np.asarray(Wk, np.float32)
    Wv = np.asarray(Wv, np.float32)
    Wo = np.asarray(Wo, np.float32)
    bo = np.asarray(bo, np.float32)

    nc = _get_program()
    tri = np.ascontiguousarray(np.triu(np.ones((128, 128), np.float32)))
    WoT = Wo.T  # (d_in, d_out)

    def wlayout(w):  # [(t 128), n] -> [128, t*n]: partition p holds t*128+p
        return np.ascontiguousarray(
            w.reshape(w.shape[0] // 128, 128, -1)
            .transpose(1, 0, 2).reshape(128, -1))

    in_maps = []
    for c in range(N_CORES):
        b, g = divmod(c, 4)
        hs = slice(g * HD, (g + 1) * HD)
        in_maps.append({
            "qT": np.ascontiguousarray(query[b].T).astype(ml_dtypes.bfloat16),
            "kT": np.ascontiguousarray(key[b].T).astype(ml_dtypes.bfloat16),
            "vT": np.ascontiguousarray(value[b].T).astype(ml_dtypes.bfloat16),
            "wq": wlayout(Wq[hs].T).astype(ml_dtypes.bfloat16),
            "wk": wlayout(Wk[hs].T).astype(ml_dtypes.bfloat16),
            "wv": wlayout(Wv[hs].T).astype(ml_dtypes.bfloat16),
            "wo": wlayout(WoT[hs]).astype(ml_dtypes.bfloat16),
            "tri": tri.astype(ml_dtypes.bfloat16),
            "ones": np.ones((128, 64), ml_dtypes.bfloat16),
        })

    global last_in_maps
    last_in_maps = in_maps
    res = run_bass_kernel_spmd(nc, in_maps, core_ids=list(range(N_CORES)))

    out = np.zeros((B, S, D), np.float32)
    for c in range(N_CORES):
        out[c // 4] += np.asarray(res.results[c]["y"], np.float32)
    out += bo
    return out


# revision 27
# speedup vs baseline: 1.0576x; 1.0089x over previous
"""Multi-head causal attention (B=2, S=2048, D=1024, H=16) on 8 NeuronCores.

Sharding: core c handles batch c//4 and head group c%4 (4 heads = 256 dims).
Wq/Wk/Wv are column-split by head; Wo is row-split; the Wo all-reduce is done
on the host by summing the 8 partial outputs (4 cores per batch), plus bo.

Key design points:
  - ALL matmuls occupy the full 128x128 PE array. The HAM clock gate only
    counts full-array activity as "busy": half-array matmuls (K=64 scores,
    M=65 attn@V) leave the PE throttled at 1.2 GHz forever. Scores use a
    zero-padded kTz (head's 64 rows in its partition half, zeros elsewhere)
    against the full 128-partition qT; attn@V uses voz padded to 128 columns
    (64 V dims + ones column + 63 zeros). Zero rows/columns add no stream
    cycles - the padding is free.
  - Phase B runs single-head streams, exp-paced on ScalarE (the hard floor:
    ~81us of exp). PSUM: scores double-buffered (4 banks) + 2 oT tiles
    (4 banks) = 8. Warm PE work per kt (~854ns) hides under exp (~1075ns).
  - attn@V deferred 2 kts behind scores so the GpSimd tri-mask multiply
    never stalls the PE.
  - The rowsum->reciprocal->broadcast chain (4 serial DMAs of latency)
    starts at its own head's end and is software-pipelined at EMISSION
    level: remaining steps are emitted at kt milestones inside the next
    head's loop (final chains inside phase C), so no in-order engine queue
    ever blocks on a chain dependency.
  - bf16 everywhere SBUF-resident (fp8 fails accuracy: quantization errors
    scale with the result, they do not average down); rowsum chain f32.
  - Input chunks round-robin across all 3 DMA-capable queues (SP/Act/Pool)
    so each projection streams at aggregate bandwidth.
  - qh=1 processed before qh=0 so phase C's qt 8-15 (which need only qh=1
    norms) stream immediately after B's last matmul.
  - PE warm-up matmuls during the initial input DMA wait (HAM ramp).
  - y written bf16 (halves output DMA); host accumulates in f32.
"""

import ml_dtypes
import numpy as np

import concourse.bass as bass
import concourse.tile as tile
from concourse import bacc, mybir
from concourse.bass_utils import run_bass_kernel_spmd

B, S, D, H = 2, 2048, 1024, 16
DH = D // H          # 64
HPC = 4              # heads per core
HD = HPC * DH        # 256 head dims per core
N_CORES = 8
DT = D // 128        # 8 contraction tiles for projections
NT = S // 128        # 16 seq tiles
F32 = mybir.dt.float32
BF16 = mybir.dt.bfloat16


def build_program():
    nc = bacc.Bacc("TRN2", target_bir_lowering=False, debug=False,
                   num_devices=N_CORES)

    qT_d = nc.dram_tensor("qT", [D, S], BF16, kind="ExternalInput").ap()
    kT_d = nc.dram_tensor("kT", [D, S], BF16, kind="ExternalInput").ap()
    vT_d = nc.dram_tensor("vT", [D, S], BF16, kind="ExternalInput").ap()
    # weights pre-transposed on host to [128, t*n] so each DMA is one
    # 4KB-contiguous line per partition (the (t p) h rearrange view has
    # 512-byte lines = 1024 descriptors = slow)
    wq_d = nc.dram_tensor("wq", [128, DT * HD], BF16,
                          kind="ExternalInput").ap()
    wk_d = nc.dram_tensor("wk", [128, DT * HD], BF16,
                          kind="ExternalInput").ap()
    wv_d = nc.dram_tensor("wv", [128, DT * HD], BF16,
                          kind="ExternalInput").ap()
    wo_d = nc.dram_tensor("wo", [128, 2 * D], BF16, kind="ExternalInput").ap()
    tri_d = nc.dram_tensor("tri", [128, 128], BF16, kind="ExternalInput").ap()
    ones_d = nc.dram_tensor("ones", [128, 64], BF16,
                            kind="ExternalInput").ap()
    # DRAM staging for rowsums and their reciprocals (partition remapping
    # and step-0 broadcasts are only possible through DRAM)
    rb_d = nc.dram_tensor("rbounce", [16, 1024], F32).ap()
    rrec_d = nc.dram_tensor("rrec", [16, 1024], F32).ap()
    y_d = nc.dram_tensor("y", [S, D], BF16, kind="ExternalOutput").ap()

    Exp = mybir.ActivationFunctionType.Exp
    DMAE = None  # set inside context

    with tile.TileContext(nc) as tc:
        dma_engs = (nc.sync, nc.gpsimd, nc.scalar)
        with (
            tc.tile_pool(name="persist", bufs=1) as persist,
            tc.tile_pool(name="qstream", bufs=6) as qstream,
            tc.tile_pool(name="kstream", bufs=6) as kstream,
            tc.tile_pool(name="vstream", bufs=6) as vstream,
            tc.tile_pool(name="pT", bufs=5) as pT_pool,
            tc.tile_pool(name="norm", bufs=2) as norm_pool,
            tc.tile_pool(name="ysb", bufs=6) as ysb_pool,
        ):
            # ---- constants / weights (single-shot DMAs, tiny first) ----
            tri_sb = persist.tile([128, 128], BF16, tag="tri")
            ones_sb = persist.tile([128, 64], BF16, tag="ones")
            nc.sync.dma_start(tri_sb, tri_d)
            nc.sync.dma_start(ones_sb, ones_d)

            wq_sb = persist.tile([128, DT, HD], BF16, tag="wq")
            wk_sb = persist.tile([128, DT, HD], BF16, tag="wk")
            wv_sb = persist.tile([128, DT, HD], BF16, tag="wv")
            wo_sb = persist.tile([128, 2, D], BF16, tag="wo")
            nc.sync.dma_start(wq_sb, wq_d.rearrange("p (t h) -> p t h", t=DT))
            nc.gpsimd.dma_start(
                wk_sb, wk_d.rearrange("p (t h) -> p t h", t=DT))
            nc.scalar.dma_start(
                wv_sb, wv_d.rearrange("p (t h) -> p t h", t=DT))

            qT_sb = persist.tile([128, 2, S], BF16, tag="qTsb")
            # kTz: per head parity e, the head's 64 kT rows sit in its
            # partition half with ZEROS in the other half -> scores contract
            # K=128 (full array) against the full-partition qT; the other
            # head's q rows are multiplied by zero weights.
            kTz_sb = persist.tile([128, 2, 2, S], BF16, tag="kTz")
            nc.vector.memset(kTz_sb[64:128, 0, :, :], 0.0)
            nc.vector.memset(kTz_sb[0:64, 1, :, :], 0.0)
            # voz: V (64 dims) + ones column (index 64, so attn@V also emits
            # the softmax row-sum as output row 64) + 63 zero columns so the
            # stationary operand is full 128 wide.
            voz_sb = persist.tile([128, NT, HPC, 128], BF16, tag="voz")
            nc.vector.memset(voz_sb[:, :, :, DH + 1:], 0.0)
            for h in range(HPC):
                nc.vector.tensor_copy(
                    voz_sb[:, :, h, DH:DH + 1],
                    ones_sb[:, 0:NT].rearrange("p (t o) -> p t o", o=1))
            aT_sb = persist.tile([128, 2, S], BF16, tag="aTsb")

            # ---- PE warm-up during initial DMA wait (HAM ramp) ----
            with tc.tile_pool(name="warm", bufs=1, space="PSUM") as warmp:
                wps = warmp.tile([128, 128], F32, tag="warm")
                for _ in range(36):
                    nc.tensor.matmul(wps, tri_sb, tri_sb,
                                     start=True, stop=True)
            # preload the exp activation table off the critical path
            dummy_pt = persist.tile([128, 16], BF16, tag="dummy")
            nc.scalar.activation(dummy_pt, tri_sb[:, 0:16], Exp, scale=0.125)

            # ---- Phase A: projections ----
            with tc.tile_pool(name="psA", bufs=8, space="PSUM") as psA:
                def proj_qk(x_d, w_sb, evac, pool):
                    ps = [psA.tile([128, 512], F32, tag="psA", name=f"psA{i}")
                          for i in range(8)]
                    for dt in range(DT):
                        for half in range(2):
                            xt = pool.tile([128, S // 2], BF16)
                            dma_engs[(dt * 2 + half) % 3].dma_start(
                                xt, x_d[dt * 128:(dt + 1) * 128,
                                        half * 1024:(half + 1) * 1024])
                            for t in range(2):
                                for c in range(2):
                                    cc = half * 2 + c
                                    nc.tensor.matmul(
                                        ps[t * 4 + cc],
                                        w_sb[:, dt, t * 128:(t + 1) * 128],
                                        xt[:, c * 512:(c + 1) * 512],
                                        start=(dt == 0), stop=(dt == DT - 1),
                                    )
                    for t in range(2):
                        for c in range(4):
                            evac(t, c, ps[t * 4 + c])

                def q_evac(t, c, src):
                    nc.vector.tensor_copy(
                        qT_sb[:, t, c * 512:(c + 1) * 512], src)

                def k_evac(t, c, src):
                    # head-even rows (0:64) -> kTz parity 0, lanes 0:64;
                    # head-odd rows (64:128) -> kTz parity 1, lanes 64:128.
                    nc.vector.tensor_copy(
                        kTz_sb[0:64, 0, t, c * 512:(c + 1) * 512],
                        src[0:64, :])
                    nc.vector.tensor_copy(
                        kTz_sb[64:128, 1, t, c * 512:(c + 1) * 512],
                        src[64:128, :])

                proj_qk(qT_d, wq_sb, q_evac, qstream)
                proj_qk(kT_d, wk_sb, k_evac, kstream)

                # V: natural layout out[m = seq_tile(128), n = head dims(256)]
                psv = [psA.tile([128, 512], F32, tag="psA", name=f"psV{i}")
                       for i in range(8)]
                for dt in range(DT):
                    for half in range(2):
                        vt = vstream.tile([128, S // 2], BF16)
                        dma_engs[(dt * 2 + half) % 3].dma_start(
                            vt, vT_d[dt * 128:(dt + 1) * 128,
                                     half * 1024:(half + 1) * 1024])
                        for nt_ in range(8):
                            ntile = half * 8 + nt_
                            nc.tensor.matmul(
                                psv[ntile // 2][:,
                                                (ntile % 2) * 256:(ntile % 2) * 256 + 256],
                                vt[:, nt_ * 128:(nt_ + 1) * 128],
                                wv_sb[:, dt, :],
                                # start only on the bank's first matmul:
                                # start=True clears has_written for the WHOLE
                                # bank; the second group must not re-clear.
                                start=(dt == 0 and ntile % 2 == 0),
                                stop=(dt == DT - 1),
                                skip_group_check=True,
                            )
                for ntile in range(NT):
                    src = psv[ntile // 2][:,
                                          (ntile % 2) * 256:(ntile % 2) * 256 + 256]
                    nc.vector.tensor_copy(
                        voz_sb[:, ntile, :, 0:DH],
                        src.rearrange("p (h d) -> p h d", h=HPC),
                    )

            # ---- Phase B: attention, single-head streams, exp-paced ----
            def make_chain(t, p64, qh0, oT, i):
                """Normalization chain for one finished head: aT[head rows]
                = oT * (1/rowsum). s1 evacuates oT (incl. the rowsum row)
                to SBUF in one DVE copy - the PSUM banks free immediately
                instead of after the whole chain, so the oT pool never
                starves the next streams. s1 is emitted at the head's own
                end; the rest at kt milestones of the next head (or phase C
                blocks) so no in-order engine queue blocks on chain DMA
                latency."""
                oc = norm_pool.tile([65, 1024], F32, tag="oc", name="oc")
                r64 = norm_pool.tile([64, 16], F32, tag="r64", name="r64")
                rec64 = norm_pool.tile([64, 16], F32, tag="rec64",
                                       name="rec64")
                rb = norm_pool.tile([64, 1024], F32, tag="rb", name="rb")
                row = rb_d[i:i + 1, :]
                rrow = rrec_d[i:i + 1, :]

                def s1():
                    nc.vector.tensor_copy(oc, oT[0:65, :])
                    nc.sync.dma_start(row, oc[64:65, :])

                def s2():
                    r64view = bass.AP(tensor=row.tensor, offset=row.offset,
                                      ap=[[16, 64], [1, 16]])
                    nc.gpsimd.dma_start(r64, r64view)

                def s3():
                    nc.vector.reciprocal(rec64, r64)
                    recback = bass.AP(tensor=rrow.tensor, offset=rrow.offset,
                                      ap=[[16, 64], [1, 16]])
                    nc.sync.dma_start(recback, rec64)

                def s4():
                    bcast = bass.AP(tensor=rrow.tensor, offset=rrow.offset,
                                    ap=[[0, 64]] + [list(rrow.ap[-1])])
                    nc.gpsimd.dma_start(rb, bcast)

                def s5():
                    if p64 == 0:
                        nc.vector.tensor_mul(
                            aT_sb[0:64, t, qh0:qh0 + 1024], oc[0:64, :], rb)
                    else:
                        stage = norm_pool.tile([64, 1024], BF16, tag="stage",
                                               name="stage")
                        nc.vector.tensor_mul(stage, oc[0:64, :], rb)
                        nc.gpsimd.dma_start(
                            aT_sb[64:128, t, qh0:qh0 + 1024], stage)

                s1()
                return [s2, s3, s4, s5]

            with (
                tc.tile_pool(name="psB", bufs=2, space="PSUM") as psB,
                tc.tile_pool(name="psO", bufs=2, space="PSUM") as psO,
            ):
                nc.sync.dma_start(
                    wo_sb, wo_d.rearrange("p (t e) -> p t e", t=2))

                chain_i = 0
                chain_ops = []  # pending emission closures from prior heads
                # qh=1 first: phase C's qt 8-15 then only wait on qh=1 norms.
                # In qh=0, heads run 3..0 so the final chain is head 0
                # (p64=0: no staging hop on the last, fully-exposed chain).
                for qh, heads in ((1, (0, 1, 2, 3)), (0, (3, 2, 1, 0))):
                    qh0 = qh * 1024
                    nkt = (qh0 + 1024) // 128
                    # kt milestones at which to emit one pending chain step
                    if nkt == 8:
                        miles = {1, 3, 5, 6}
                    else:
                        miles = {2, 5, 8, 11}
                    for hh in heads:
                        t, e = hh // 2, hh % 2
                        p64 = e * 64
                        oT = psO.tile([128, 1024], F32, tag="oT", name="oT")

                        def attn_v(kt, pT_t, qs, oT=oT, qh0=qh0, hh=hh):
                            for qc in range(2):
                                c0 = qh0 + qc * 512
                                lo = max(qs, c0)
                                if lo >= c0 + 512:
                                    continue
                                ktl = (c0 + 512) // 128 - 1
                                nc.tensor.matmul(
                                    oT[:, qc * 512 + (lo - c0):(qc + 1) * 512],
                                    voz_sb[:, kt, hh, :],
                                    pT_t[:, lo - qh0:c0 + 512 - qh0],
                                    start=(kt == 0), stop=(kt == ktl),
                                )

                        pending = []
                        for kt in range(nkt):
                            k0 = kt * 128
                            qs = max(k0, qh0)
                            off = qs - qh0
                            st = psB.tile([128, 1024], F32, tag="st",
                                          name="st")
                            for bank in range(2):
                                glo = max(qs, qh0 + bank * 512)
                                ghi = qh0 + (bank + 1) * 512
                                if glo < ghi:
                                    nc.tensor.matmul(
                                        st[:, glo - qh0:ghi - qh0],
                                        kTz_sb[:, e, t, k0:k0 + 128],
                                        qT_sb[:, t, glo:ghi],
                                        start=True, stop=True,
                                    )
                            pT_t = pT_pool.tile([128, 1024], BF16, tag="pT",
                                                name="pT")
                            nc.scalar.activation(
                                pT_t[:, off:1024], st[:, off:1024], Exp,
                                scale=0.125)
                            if k0 >= qh0:
                                nc.gpsimd.tensor_mul(
                                    pT_t[:, off:off + 128],
                                    pT_t[:, off:off + 128], tri_sb)
                            pending.append((kt, pT_t, qs))
                            if len(pending) > 2:
                                attn_v(*pending.pop(0))
                            if kt in miles and chain_ops:
                                chain_ops.pop(0)()
                        for p_ in pending:
                            attn_v(*p_)
                        # flush any chain debt before enqueueing this head's
                        for op in chain_ops:
                            op()
                        chain_ops = make_chain(t, p64, qh0, oT, chain_i)
                        chain_i += 1

                # ---- Phase C: output projection, interleaved with the
                # ---- final head's chain. qt 8-15 first (they only need
                # ---- qh=1 norms, which completed long ago). PSUM comes
                # ---- from the scores pool (tag st) - no spare banks.
                for i, qt in enumerate(list(range(8, NT)) + list(range(8))):
                    if i in (0, 1, 2, 3) and chain_ops:
                        chain_ops.pop(0)()
                    for e in range(2):
                        py = psB.tile([128, 1024], F32, tag="st", name="py")
                        for t in range(2):
                            nc.tensor.matmul(
                                py[:, 0:512],
                                aT_sb[:, t, qt * 128:(qt + 1) * 128],
                                wo_sb[:, t, e * 512:(e + 1) * 512],
                                start=(t == 0), stop=(t == 1),
                            )
                        ot = ysb_pool.tile([128, 512], BF16, tag="ysb",
                                           name="ysb")
                        if (i * 2 + e) % 2 == 0:
                            nc.vector.tensor_copy(ot, py[:, 0:512])
                        else:
                            nc.scalar.copy(ot, py[:, 0:512])
                        dma_eng = dma_engs[(i * 2 + e) % 3]
                        dma_eng.dma_start(
                            y_d[qt * 128:(qt + 1) * 128,
                                e * 512:(e + 1) * 512], ot)

    nc.compile()
    return nc


_CACHE = {}
last_in_maps = None


def _get_program():
    if "nc" not in _CACHE:
        _CACHE["nc"] = build_program()
    return _CACHE["nc"]


def kernel(query, key, value, mask, Wq, Wk, Wv, Wo, bo):
    query = np.asarray(query, np.float32)
    key = np.asarray(key, np.float32)
    value = np.asarray(value, np.float32)
    Wq = np.asarray(Wq, np.float32)
    Wk = np.asarray(Wk, np.float32)
    Wv = np.asarray(Wv, np.float32)
    Wo = np.asarray(Wo, np.float32)
    bo = np.asarray(bo, np.float32)

    nc = _get_program()
    tri = np.ascontiguousarray(np.triu(np.ones((128, 128), np.float32)))
    WoT = Wo.T  # (d_in, d_out)

    def wlayout(w):  # [(t 128), n] -> [128, t*n]: partition p holds t*128+p
        return np.ascontiguousarray(
            w.reshape(w.shape[0] // 128, 128, -1)
            .transpose(1, 0, 2).reshape(128, -1))

    in_maps = []
    for c in range(N_CORES):
        b, g = divmod(c, 4)
        hs = slice(g * HD, (g + 1) * HD)
        in_maps.append({
            "qT": np.ascontiguousarray(query[b].T).astype(ml_dtypes.bfloat16),
            "kT": np.ascontiguousarray(key[b].T).astype(ml_dtypes.bfloat16),
            "vT": np.ascontiguousarray(value[b].T).astype(ml_dtypes.bfloat16),
            "wq": wlayout(Wq[hs].T).astype(ml_dtypes.bfloat16),
            "wk": wlayout(Wk[hs].T).astype(ml_dtypes.bfloat16),
            "wv": wlayout(Wv[hs].T).astype(ml_dtypes.bfloat16),
            "wo": wlayout(WoT[hs]).astype(ml_dtypes.bfloat16),
            "tri": tri.astype(ml_dtypes.bfloat16),
            "ones": np.ones((128, 64), ml_dtypes.bfloat16),
        })

    global last_in_maps
    last_in_maps = in_maps
    res = run_bass_kernel_spmd(nc, in_maps, core_ids=list(range(N_CORES)))

    out = np.zeros((B, S, D), np.float32)
    for c in range(N_CORES):
        out[c // 4] += np.asarray(res.results[c]["y"], np.float32)
    out += bo
    return out
